# revision 12
# baseline (speedup 1.0000x reference)
"""Fused GroupNorm + legacy-split multi-head attention + 1x1 projection with
residual, for x:(2, 256, 64, 64), on 8 Trainium2 NeuronCores.

Sharding: core i = 4*b + j handles batch b and t-slice j (1024 of the 4096
flattened spatial positions). k/v are computed for the full sequence on every
core of a batch group (cheap, redundant); each core's projection output slice
is complete, so the host only concatenates slices — no collectives.

SPMD: all cores run the identical program. The host rotates each core's copy
of x along t so the core's own slice sits at columns 0:1024 (GroupNorm stats
and the attention contraction over s are invariant to a consistent
permutation of the contracted axis).

Math layout notes:
- scores are computed transposed, S^T[s, t] = k^T q, so softmax's reduction
  runs over the PSUM partition dim; the denominator comes for free from a
  ones-column appended to v^T in the a = w v matmul (output row 64).
- no max-subtraction in softmax: scores are ~N(0, 1) with |s| < ~8, exp is
  safe in fp32 (verified against the reference on host).
- q/k biases are added during the PSUM->SBUF copies; the v bias is folded
  into the projection bias on the host; the attention scale 1/sqrt(ch) is
  folded into exp's scale argument.
- matmuls run in float32r (~1.2e-4 relative rounding, full PE speed).
"""
import math
from contextlib import ExitStack

import numpy as np

import concourse.bacc as bacc
import concourse.tile as tile
from concourse import mybir
from concourse.bass_utils import run_bass_kernel_spmd

f32 = mybir.dt.float32
f32r = mybir.dt.float32r
f16 = mybir.dt.float16
FT = mybir.ActivationFunctionType
ALU = mybir.AluOpType

B, C, HH, WW = 2, 256, 64, 64
T = HH * WW           # 4096
TS = T // 4           # 1024 t-columns per core
HEADS = 4
CH = C // HEADS       # 64 channels per head
NT = TS // 512        # 512-wide matmul output tiles per t-slice
SJ = T // 128         # 32 s-tiles
EPS = 1e-5
N_CORES = 8
EXP_SCALE = 1.0 / math.sqrt(CH)  # (1/ch^0.25)^2 folded into exp

_CACHE: dict = {}


def _build():
    nc = bacc.Bacc("TRN2", target_bir_lowering=False, debug=False,
                   num_devices=N_CORES)

    def dram_in(name, shape, dtype=f32):
        return nc.dram_tensor(name, shape, dtype, kind="ExternalInput").ap()

    x = dram_in("x", [C, T])
    qwt = dram_in("qwt", [C, C], f16)
    kwt = dram_in("kwt", [C, C], f16)
    vwt = dram_in("vwt", [C, C], f16)
    pwt = dram_in("pwt", [C, C], f16)
    qb2 = dram_in("qb2", [128, 2])
    kb2 = dram_in("kb2", [128, 2])
    pb2 = dram_in("pb2", [128, 2])
    nw2 = dram_in("nw2", [128, 2])
    nb2 = dram_in("nb2", [128, 2])
    gsel = dram_in("gsel", [128, 16], f32r)
    gselt = dram_in("gselt", [16, 128], f32r)
    ones = dram_in("ones", [128, 128], f16)
    zer = dram_in("zer", [128, 65], f16)
    out = nc.dram_tensor("out", [C, TS], f32, kind="ExternalOutput").ap()

    x2 = x.rearrange("(i p) t -> p i t", i=2)  # [128, 2, 4096] view

    with tile.TileContext(nc) as tc, ExitStack() as ctx:
        sb1 = ctx.enter_context(tc.tile_pool(name="sb1", bufs=1))
        wp = ctx.enter_context(tc.tile_pool(name="wp", bufs=4))
        st = ctx.enter_context(tc.tile_pool(name="st", bufs=2))
        rp = ctx.enter_context(tc.tile_pool(name="rp", bufs=2))
        ps = ctx.enter_context(tc.tile_pool(name="ps", bufs=1, space="PSUM"))
        psa = ctx.enter_context(tc.tile_pool(name="psa", bufs=1, space="PSUM"))

        # ---- persistent tiles ----
        x_sb = sb1.tile([128, 2, T], f32)
        _qs = (nc.sync, nc.gpsimd, nc.scalar)
        for n, (i, c2) in enumerate([(i, c2) for i in range(2) for c2 in range(4)]):
            _qs[n % 3].dma_start(out=x_sb[:, i, c2 * 1024:(c2 + 1) * 1024],
                                 in_=x2[:, i, c2 * 1024:(c2 + 1) * 1024])
        qwt_sb = sb1.tile([128, 2, C], f16)
        kwt_sb = sb1.tile([128, 2, C], f16)
        vwt_sb = sb1.tile([128, 2, C], f16)
        pwt_sb = sb1.tile([128, 2, C], f16)
        for dst, src in ((qwt_sb, qwt), (kwt_sb, kwt), (vwt_sb, vwt), (pwt_sb, pwt)):
            nc.scalar.dma_start(out=dst[:], in_=src.rearrange("(i p) o -> p i o", i=2))
        qb_sb = sb1.tile([128, 2], f32)
        kb_sb = sb1.tile([128, 2], f32)
        pb_sb = sb1.tile([128, 2], f32)
        nw_sb = sb1.tile([128, 2], f32)
        nb_sb = sb1.tile([128, 2], f32)
        for dst, src in ((qb_sb, qb2), (kb_sb, kb2), (pb_sb, pb2), (nw_sb, nw2), (nb_sb, nb2)):
            nc.scalar.dma_start(out=dst[:], in_=src[:])
        gsel_sb = sb1.tile([128, 16], f32r)
        nc.scalar.dma_start(out=gsel_sb[:], in_=gsel[:])
        gselt_sb = sb1.tile([16, 128], f32r)
        nc.scalar.dma_start(out=gselt_sb[:], in_=gselt[:])
        ones_sb = sb1.tile([128, 128], f16)
        nc.scalar.dma_start(out=ones_sb[:], in_=ones[:])
        zer_sb = sb1.tile([128, 65], f16)
        nc.scalar.dma_start(out=zer_sb[:], in_=zer[:])
        eps_sb = sb1.tile([128, 1], f32)
        nc.vector.memset(eps_sb[:], EPS)

        xn = sb1.tile([128, 2, T], f16)
        k_sb = sb1.tile([128, 2, T], f16)
        q_sb = sb1.tile([128, 2, TS], f16)
        vaug = sb1.tile([128, SJ, HEADS, CH + 1], f16)
        a_sb = sb1.tile([128, 2, TS], f16)

        # ones column of vaug (col CH of every (j, h) slot)
        nc.vector.tensor_copy(
            out=vaug[:, :, :, CH:CH + 1],
            in_=ones_sb[:, 0:SJ * HEADS].rearrange("p (j h) -> p j h", j=SJ),
        )

        # ---- phase A: GroupNorm ----
        stats_all = sb1.tile([128, 2, 8, 6], f32)
        ab = []  # per c-tile (alpha, beta) [128, 2]
        for i in range(2):
            for s in range(8):
                nc.vector.bn_stats(
                    out=stats_all[:, i, s, :],
                    in_=x_sb[:, i, s * 512:(s + 1) * 512],
                )
            hp = tc.high_priority()
            hp.__enter__()
            mv = st.tile([128, 2], f32, name=f"mv_{i}", tag="mv")
            nc.vector.bn_aggr(out=mv[:], in_=stats_all[:, i])
            # me = (mean_c, E[x^2]_c)
            me = st.tile([128, 2], f32, name=f"me_{i}", tag="me")
            nc.vector.tensor_copy(out=me[:, 0:1], in_=mv[:, 0:1])
            nc.vector.tensor_tensor(out=me[:, 1:2], in0=mv[:, 0:1], in1=mv[:, 0:1], op=ALU.mult)
            nc.vector.tensor_add(out=me[:, 1:2], in0=me[:, 1:2], in1=mv[:, 1:2])
            me_r = st.tile([128, 2], f32r, name=f"me_r_{i}", tag="me_r")
            nc.vector.tensor_copy(out=me_r[:], in_=me[:])
            # group sums: [16, 2] = sum over the 8 channels of each group
            gs_ps = ps.tile([16, 2], f32, name=f"gs_ps_{i}", tag="sc0")
            nc.tensor.matmul(out=gs_ps[:], lhsT=gsel_sb[:], rhs=me_r[:], start=True, stop=True)
            gstats = st.tile([16, 2], f32, name=f"gstats_{i}", tag="gstats")
            nc.vector.tensor_scalar_mul(out=gstats[:], in0=gs_ps[:], scalar1=1.0 / 8.0)
            tmp1 = st.tile([16, 1], f32, name=f"tmp1_{i}", tag="tmp1")
            nc.vector.tensor_tensor(out=tmp1[:], in0=gstats[:, 0:1], in1=gstats[:, 0:1], op=ALU.mult)
            nc.vector.tensor_sub(out=gstats[:, 1:2], in0=gstats[:, 1:2], in1=tmp1[:])
            nc.scalar.activation(out=gstats[:, 1:2], in_=gstats[:, 1:2], func=FT.Sqrt,
                                 bias=eps_sb[0:16, :])
            nc.vector.reciprocal(out=gstats[:, 1:2], in_=gstats[:, 1:2])
            gstats_r = st.tile([16, 2], f32r, name=f"gstats_r_{i}", tag="gstats_r")
            nc.vector.tensor_copy(out=gstats_r[:], in_=gstats[:])
            # broadcast to channels: [128, 2] = (mean_c, rstd_c)
            ch_ps = ps.tile([128, 2], f32, name=f"ch_ps_{i}", tag="sc1")
            nc.tensor.matmul(out=ch_ps[:], lhsT=gselt_sb[:], rhs=gstats_r[:], start=True, stop=True)
            ab_i = st.tile([128, 2], f32, name=f"ab_{i}", tag="ab", bufs=2)
            nc.vector.tensor_tensor(out=ab_i[:, 0:1], in0=ch_ps[:, 1:2], in1=nw_sb[:, i:i + 1], op=ALU.mult)
            tmp2 = st.tile([128, 1], f32, name=f"tmp2_{i}", tag="tmp2")
            nc.vector.tensor_tensor(out=tmp2[:], in0=ch_ps[:, 0:1], in1=ab_i[:, 0:1], op=ALU.mult)
            nc.vector.tensor_sub(out=ab_i[:, 1:2], in0=nb_sb[:, i:i + 1], in1=tmp2[:])
            hp.__exit__(None, None, None)
            ab.append(ab_i)

        # apply affine -> xn (f16): c-tile 0 on DVE, c-tile 1 on ACT
        for i in range(2):
            for c2 in range(2):
                if i == 0:
                    nc.vector.tensor_scalar(
                        out=xn[:, i, c2 * 2048:(c2 + 1) * 2048],
                        in0=x_sb[:, i, c2 * 2048:(c2 + 1) * 2048],
                        scalar1=ab[i][:, 0:1], scalar2=ab[i][:, 1:2],
                        op0=ALU.mult, op1=ALU.add,
                    )
                else:
                    nc.scalar.activation(
                        out=xn[:, i, c2 * 2048:(c2 + 1) * 2048],
                        in_=x_sb[:, i, c2 * 2048:(c2 + 1) * 2048],
                        func=FT.Identity,
                        scale=ab[i][:, 0:1], bias=ab[i][:, 1:2],
                    )

        # ---- phase B: qkv projections ----
        # PE warm-up: ~5us of dummy matmuls so HAM reaches K=8/8 before the
        # dense qkv/attention stream (output is discarded).
        warm_ps = ps.tile([128, 512], f32, name="warm_ps", tag="sc0")
        for _ in range(48):
            nc.tensor.matmul(out=warm_ps[:, 0:128], lhsT=ones_sb[:], rhs=ones_sb[:],
                             start=True, stop=True)
        # q: [128, 2(pair), 1024]
        for p in range(2):
            q_ps = ps.tile([128, TS], f32, name=f"q_ps_{p}", tag=f"sc{p}")
            for nt in range(NT):
                for i in range(2):
                    nc.tensor.matmul(
                        out=q_ps[:, nt * 512:(nt + 1) * 512],
                        lhsT=qwt_sb[:, i, p * 128:(p + 1) * 128],
                        rhs=xn[:, i, nt * 512:(nt + 1) * 512],
                        start=(i == 0), stop=(i == 1),
                    )
            nc.vector.tensor_scalar_add(out=q_sb[:, p, :], in0=q_ps[:], scalar1=qb_sb[:, p:p + 1])
        # k and v^T production interleaved with attention consumption:
        # after chunk c4's k/v^T are emitted, attention js of chunk c4-1 for
        # pair 0 run, keeping ACT (exp) continuously busy from ~40us on.
        def kv_chunk_thunks(c4):
            """Emission thunks for chunk c4's k and v^T, to be spread between
            attention iterations (keeps the exp stream gap-free)."""
            thunks = []
            for p in range(2):
                def mk_k(p=p):
                    k_ps = ps.tile([128, 1024], f32, name=f"k_ps_{p}_{c4}", tag=f"sc{p}")
                    for nt in range(2):
                        for i in range(2):
                            nc.tensor.matmul(
                                out=k_ps[:, nt * 512:(nt + 1) * 512],
                                lhsT=kwt_sb[:, i, p * 128:(p + 1) * 128],
                                rhs=xn[:, i, c4 * 1024 + nt * 512: c4 * 1024 + (nt + 1) * 512],
                                start=(i == 0), stop=(i == 1),
                            )
                    nc.vector.tensor_scalar_add(
                        out=k_sb[:, p, c4 * 1024:(c4 + 1) * 1024], in0=k_ps[:],
                        scalar1=kb_sb[:, p:p + 1],
                    )
                thunks.append(mk_k)
            for j in range(8 * c4, 8 * c4 + 8):
                def mk_v(j=j):
                    vt_ps = ps.tile([128, C], f32, name=f"vt_ps_{j}", tag=f"sc{j % 2}")
                    for i in range(2):
                        nc.tensor.matmul(
                            out=vt_ps[:], lhsT=xn[:, i, j * 128:(j + 1) * 128],
                            rhs=vwt_sb[:, i, :], start=(i == 0), stop=(i == 1),
                        )
                    nc.vector.tensor_copy(
                        out=vaug[:, j, :, 0:CH],
                        in_=vt_ps.rearrange("p (h c) -> p h c", h=HEADS),
                    )
                thunks.append(mk_v)
            return thunks

        att = {}  # per-pair attention state: (avs, prev_w)
        att = {}  # per-pair attention state: (avs, prev_w)

        def emit_att(p, js, side=None):
            avs, prev_w = att[p]
            side = list(side or [])
            si = 0
            per_j = max(1, (len(side) + len(js) - 1) // len(js)) if side else 0
            for j in js:
                cur_w = [None, None]
                for hh in range(2):
                    h = 2 * p + hh
                    off = hh * CH
                    s_ps = ps.tile([128, TS], f32, name=f"s_ps_{h}_{j}", tag=f"sc{hh}")
                    for nt in range(NT):
                        nc.tensor.matmul(
                            out=s_ps[:, nt * 512:(nt + 1) * 512],
                            lhsT=k_sb[off:off + CH, p, j * 128:(j + 1) * 128],
                            rhs=q_sb[off:off + CH, p, nt * 512:(nt + 1) * 512],
                            start=True, stop=True,
                        )
                    w_t = wp.tile([128, TS], f16, name=f"w_{h}_{j}", tag="w")
                    cur_w[hh] = w_t
                    nc.scalar.activation(out=w_t[:], in_=s_ps[:], func=FT.Exp,
                                         scale=EXP_SCALE)
                    if prev_w[hh] is not None:
                        for nt in range(NT):
                            nc.tensor.matmul(
                                out=avs[hh][:, nt * 512:(nt + 1) * 512],
                                lhsT=vaug[:, j - 1, h, :],
                                rhs=prev_w[hh][:, nt * 512:(nt + 1) * 512],
                                start=(j - 1 == 0), stop=False,
                            )
                if side and si < len(side):
                    for t in side[si:si + per_j]:
                        t()
                    si += per_j
                elif 2 <= j < SJ - 1:
                    for f in range(2):
                        nc.tensor.matmul(
                            out=avs[0][:, f * 512:(f + 1) * 512],
                            lhsT=zer_sb[:], rhs=xn[:, 0, 0:512],
                            start=False, stop=False, skip_group_check=True,
                        )
                prev_w = cur_w
            for t in side[si:]:
                t()
            att[p] = (avs, prev_w)

        def finish_pair(p):
            avs, prev_w = att[p]
            for hh in range(2):
                h = 2 * p + hh
                for nt in range(NT):
                    nc.tensor.matmul(
                        out=avs[hh][:, nt * 512:(nt + 1) * 512],
                        lhsT=vaug[:, SJ - 1, h, :],
                        rhs=prev_w[hh][:, nt * 512:(nt + 1) * 512],
                        start=False, stop=True,
                    )
            for hh in range(2):
                h = 2 * p + hh
                off = hh * CH
                av = avs[hh]
                # normalize: a = av[0:CH] / av[CH]
                d16 = rp.tile([1, TS], f16, name=f"d16_{h}", tag="d16")
                nc.vector.tensor_copy(out=d16[:], in_=av[CH:CH + 1, :])
                drep_ps = ps.tile([CH, TS], f32, name=f"drep_ps_{h}", tag=f"sc{hh}")
                for nt in range(NT):
                    nc.tensor.matmul(
                        out=drep_ps[:, nt * 512:(nt + 1) * 512],
                        lhsT=ones_sb[0:1, 0:CH],
                        rhs=d16[:, nt * 512:(nt + 1) * 512],
                        start=True, stop=True,
                    )
                rrep_sb = rp.tile([CH, TS], f32, name=f"rrep_sb_{h}", tag="rrep")
                nc.vector.reciprocal_approx_fast(out=rrep_sb[:], in_=drep_ps[:])
                nc.vector.tensor_tensor(
                    out=a_sb[off:off + CH, p, :], in0=av[0:CH, :], in1=rrep_sb[:],
                    op=ALU.mult,
                )

        att[0] = ([psa.tile([CH + 1, TS], f32, name=f"av_{hh}", tag=f"acc{hh}")
                   for hh in range(2)], [None, None])
        for t in kv_chunk_thunks(0):
            t()
        for c4 in range(1, 4):
            emit_att(0, range(8 * (c4 - 1), 8 * c4), side=kv_chunk_thunks(c4))
        emit_att(0, range(24, SJ))
        finish_pair(0)
        att[1] = ([psa.tile([CH + 1, TS], f32, name=f"av_{2 + hh}", tag=f"acc{hh}")
                   for hh in range(2)], [None, None])
        emit_att(1, range(SJ))
        finish_pair(1)

        # ---- phase D: projection + residual ----
        for m in range(2):
            h_ps = ps.tile([128, TS], f32, name=f"h_ps_{m}", tag=f"sc{m}")
            for nt in range(NT):
                for i in range(2):
                    nc.tensor.matmul(
                        out=h_ps[:, nt * 512:(nt + 1) * 512],
                        lhsT=pwt_sb[:, i, m * 128:(m + 1) * 128],
                        rhs=a_sb[:, i, nt * 512:(nt + 1) * 512],
                        start=(i == 0), stop=(i == 1),
                    )
            o_sb = wp.tile([128, TS], f32, name=f"o_sb_{m}", tag="w")
            nc.vector.scalar_tensor_tensor(
                out=o_sb[:], in0=h_ps[:], scalar=pb_sb[:, m:m + 1], in1=x_sb[:, m, 0:TS],
                op0=ALU.add, op1=ALU.add,
            )
            nc.sync.dma_start(out=out[m * 128:(m + 1) * 128, :], in_=o_sb[:])

    nc.compile()
    return nc


def _host_inputs(x, norm_w, norm_b, qkv_w, qkv_b, proj_w, proj_b):
    """Build the 8 per-core input maps (all float32 numpy)."""
    x = np.ascontiguousarray(np.asarray(x, dtype=np.float32)).reshape(B, C, T)
    norm_w = np.asarray(norm_w, dtype=np.float32)
    norm_b = np.asarray(norm_b, dtype=np.float32)
    qkv_w = np.asarray(qkv_w, dtype=np.float32)
    qkv_b = np.asarray(qkv_b, dtype=np.float32)
    proj_w = np.asarray(proj_w, dtype=np.float32)
    proj_b = np.asarray(proj_b, dtype=np.float32)

    # head-major row gathers of the qkv conv
    q_rows = np.concatenate([np.arange(192 * h, 192 * h + 64) for h in range(HEADS)])
    k_rows = q_rows + 64
    v_rows = q_rows + 128
    qwt = np.ascontiguousarray(qkv_w[q_rows].T.astype(np.float16))
    kwt = np.ascontiguousarray(qkv_w[k_rows].T.astype(np.float16))
    vwt = np.ascontiguousarray(qkv_w[v_rows].T.astype(np.float16))
    pwt = np.ascontiguousarray(proj_w.T.astype(np.float16))

    def as2(v):  # (256,) -> [128, 2] with column p = channels 128p..128p+128
        return np.ascontiguousarray(v.reshape(2, 128).T)

    qb2 = as2(qkv_b[q_rows])
    kb2 = as2(qkv_b[k_rows])
    # v bias folded into projection bias (a_norm lacks +vb; h += proj_w @ vb)
    vb_nat = qkv_b[v_rows]  # natural channel order == head-major for v
    pb2 = as2(proj_b + proj_w @ vb_nat)
    nw2 = as2(norm_w)
    nb2 = as2(norm_b)

    gsel = np.zeros((128, 16), np.float32)
    gsel[np.arange(128), np.arange(128) // 8] = 1.0
    gselt = np.ascontiguousarray(gsel.T)
    ones = np.ones((128, 128), np.float16)

    shared = dict(qwt=qwt, kwt=kwt, vwt=vwt, pwt=pwt, qb2=qb2, kb2=kb2,
                  pb2=pb2, nw2=nw2, nb2=nb2, gsel=gsel, gselt=gselt, ones=ones,
                  zer=np.zeros((128, 65), np.float16))
    in_maps = []
    for core in range(N_CORES):
        b, j = core // 4, core % 4
        xr = np.concatenate([x[b][:, j * TS:], x[b][:, :j * TS]], axis=1)
        in_maps.append({"x": np.ascontiguousarray(xr), **shared})
    return in_maps


def _run(in_maps, **kw):
    if "nc" not in _CACHE:
        _CACHE["nc"] = _build()
    return run_bass_kernel_spmd(_CACHE["nc"], in_maps, list(range(N_CORES)), **kw)


def kernel(x, norm_w, norm_b, qkv_w, qkv_b, proj_w, proj_b):
    in_maps = _host_inputs(x, norm_w, norm_b, qkv_w, qkv_b, proj_w, proj_b)
    res = _run(in_maps)
    out = np.empty((B, C, T), np.float32)
    for core in range(N_CORES):
        b, j = core // 4, core % 4
        out[b][:, j * TS:(j + 1) * TS] = res.results[core]["out"]
    return out.reshape(B, C, HH, WW)


# revision 13
# speedup vs baseline: 1.1635x; 1.1635x over previous
"""Fused GroupNorm + legacy-split multi-head attention + 1x1 projection with
residual, for x:(2, 256, 64, 64), on 8 Trainium2 NeuronCores.

Sharding: core i = 4*b + j handles batch b and t-slice j (1024 of the 4096
flattened spatial positions). k/v are computed for the full sequence on every
core of a batch group (cheap, redundant); each core's projection output slice
is complete, so the host only concatenates slices — no collectives.

SPMD: all cores run the identical program. The host rotates each core's copy
of x along t so the core's own slice sits at columns 0:1024 (GroupNorm stats
and the attention contraction over s are invariant to a consistent
permutation of the contracted axis).

Math layout notes:
- scores are computed transposed, S^T[s, t] = k^T q, so softmax's reduction
  runs over the PSUM partition dim; the denominator comes for free from a
  ones-column appended to v^T in the a = w v matmul (output row 64).
- no max-subtraction in softmax: scores are ~N(0, 1) with |s| < ~8, exp is
  safe in fp32 (verified against the reference on host).
- q/k biases are added during the PSUM->SBUF copies; the v bias is folded
  into the projection bias on the host; the attention scale 1/sqrt(ch) is
  folded into exp's scale argument.
- matmuls run in float32r (~1.2e-4 relative rounding, full PE speed).
"""
import math
from contextlib import ExitStack

import numpy as np

import concourse.bacc as bacc
import concourse.tile as tile
from concourse import mybir
from concourse.bass_utils import run_bass_kernel_spmd

f32 = mybir.dt.float32
f32r = mybir.dt.float32r
f16 = mybir.dt.float16
FT = mybir.ActivationFunctionType
ALU = mybir.AluOpType

B, C, HH, WW = 2, 256, 64, 64
T = HH * WW           # 4096
TS = T // 4           # 1024 t-columns per core
HEADS = 4
CH = C // HEADS       # 64 channels per head
NT = TS // 512        # 512-wide matmul output tiles per t-slice
SJ = T // 128         # 32 s-tiles
EPS = 1e-5
N_CORES = 8
EXP_SCALE = 1.0 / math.sqrt(CH)  # (1/ch^0.25)^2 folded into exp

_CACHE: dict = {}


def _build():
    nc = bacc.Bacc("TRN2", target_bir_lowering=False, debug=False,
                   num_devices=N_CORES)

    def dram_in(name, shape, dtype=f32):
        return nc.dram_tensor(name, shape, dtype, kind="ExternalInput").ap()

    x = dram_in("x", [C, T])
    qwt = dram_in("qwt", [C, C], f16)
    kwt = dram_in("kwt", [C, C], f16)
    vwt = dram_in("vwt", [C, C], f16)
    pwt = dram_in("pwt", [C, C], f16)
    qb2 = dram_in("qb2", [128, 2])
    kb2 = dram_in("kb2", [128, 2])
    pb2 = dram_in("pb2", [128, 2])
    nw2 = dram_in("nw2", [128, 2])
    nb2 = dram_in("nb2", [128, 2])
    gsel = dram_in("gsel", [128, 16], f32r)
    gselt = dram_in("gselt", [16, 128], f32r)
    ones = dram_in("ones", [128, 128], f16)
    zer = dram_in("zer", [128, 65], f16)
    out = nc.dram_tensor("out", [C, TS], f32, kind="ExternalOutput").ap()

    x2 = x.rearrange("(i p) t -> p i t", i=2)  # [128, 2, 4096] view

    with tile.TileContext(nc) as tc, ExitStack() as ctx:
        sb1 = ctx.enter_context(tc.tile_pool(name="sb1", bufs=1))
        wp = ctx.enter_context(tc.tile_pool(name="wp", bufs=4))
        st = ctx.enter_context(tc.tile_pool(name="st", bufs=2))
        rp = ctx.enter_context(tc.tile_pool(name="rp", bufs=2))
        ps = ctx.enter_context(tc.tile_pool(name="ps", bufs=1, space="PSUM"))
        psa = ctx.enter_context(tc.tile_pool(name="psa", bufs=1, space="PSUM"))

        # ---- persistent tiles ----
        x_sb = sb1.tile([128, 2, T], f32)
        _qs = (nc.sync, nc.gpsimd, nc.scalar)
        for n, (i, c2) in enumerate([(i, c2) for i in range(2) for c2 in range(4)]):
            _qs[n % 3].dma_start(out=x_sb[:, i, c2 * 1024:(c2 + 1) * 1024],
                                 in_=x2[:, i, c2 * 1024:(c2 + 1) * 1024])
        qwt_sb = sb1.tile([128, 2, C], f16)
        kwt_sb = sb1.tile([128, 2, C], f16)
        vwt_sb = sb1.tile([128, 2, C], f16)
        pwt_sb = sb1.tile([128, 2, C], f16)
        for dst, src in ((qwt_sb, qwt), (kwt_sb, kwt), (vwt_sb, vwt), (pwt_sb, pwt)):
            nc.scalar.dma_start(out=dst[:], in_=src.rearrange("(i p) o -> p i o", i=2))
        qb_sb = sb1.tile([128, 2], f32)
        kb_sb = sb1.tile([128, 2], f32)
        pb_sb = sb1.tile([128, 2], f32)
        nw_sb = sb1.tile([128, 2], f32)
        nb_sb = sb1.tile([128, 2], f32)
        for dst, src in ((qb_sb, qb2), (kb_sb, kb2), (pb_sb, pb2), (nw_sb, nw2), (nb_sb, nb2)):
            nc.scalar.dma_start(out=dst[:], in_=src[:])
        gsel_sb = sb1.tile([128, 16], f32r)
        nc.scalar.dma_start(out=gsel_sb[:], in_=gsel[:])
        gselt_sb = sb1.tile([16, 128], f32r)
        nc.scalar.dma_start(out=gselt_sb[:], in_=gselt[:])
        ones_sb = sb1.tile([128, 128], f16)
        nc.scalar.dma_start(out=ones_sb[:], in_=ones[:])
        zer_sb = sb1.tile([128, 65], f16)
        nc.scalar.dma_start(out=zer_sb[:], in_=zer[:])
        eps_sb = sb1.tile([128, 1], f32)
        nc.vector.memset(eps_sb[:], EPS)

        xn = sb1.tile([128, 2, T], f16)
        k_sb = sb1.tile([128, 2, T], f16)
        q_sb = sb1.tile([128, 2, TS], f16)
        vaug = sb1.tile([128, SJ, HEADS, CH + 1], f16)
        a_sb = sb1.tile([128, 2, TS], f16)

        # ones column of vaug (col CH of every (j, h) slot)
        nc.vector.tensor_copy(
            out=vaug[:, :, :, CH:CH + 1],
            in_=ones_sb[:, 0:SJ * HEADS].rearrange("p (j h) -> p j h", j=SJ),
        )

        # ---- phase A: GroupNorm ----
        stats_all = sb1.tile([128, 2, 8, 6], f32)
        ab = []  # per c-tile (alpha, beta) [128, 2]
        for i in range(2):
            for s in range(8):
                nc.vector.bn_stats(
                    out=stats_all[:, i, s, :],
                    in_=x_sb[:, i, s * 512:(s + 1) * 512],
                )
            hp = tc.high_priority()
            hp.__enter__()
            mv = st.tile([128, 2], f32, name=f"mv_{i}", tag="mv")
            nc.vector.bn_aggr(out=mv[:], in_=stats_all[:, i])
            # me = (mean_c, E[x^2]_c)
            me = st.tile([128, 2], f32, name=f"me_{i}", tag="me")
            nc.vector.tensor_copy(out=me[:, 0:1], in_=mv[:, 0:1])
            nc.vector.tensor_tensor(out=me[:, 1:2], in0=mv[:, 0:1], in1=mv[:, 0:1], op=ALU.mult)
            nc.vector.tensor_add(out=me[:, 1:2], in0=me[:, 1:2], in1=mv[:, 1:2])
            me_r = st.tile([128, 2], f32r, name=f"me_r_{i}", tag="me_r")
            nc.vector.tensor_copy(out=me_r[:], in_=me[:])
            # group sums: [16, 2] = sum over the 8 channels of each group
            gs_ps = ps.tile([16, 2], f32, name=f"gs_ps_{i}", tag="sc0")
            nc.tensor.matmul(out=gs_ps[:], lhsT=gsel_sb[:], rhs=me_r[:], start=True, stop=True)
            gstats = st.tile([16, 2], f32, name=f"gstats_{i}", tag="gstats")
            nc.vector.tensor_scalar_mul(out=gstats[:], in0=gs_ps[:], scalar1=1.0 / 8.0)
            tmp1 = st.tile([16, 1], f32, name=f"tmp1_{i}", tag="tmp1")
            nc.vector.tensor_tensor(out=tmp1[:], in0=gstats[:, 0:1], in1=gstats[:, 0:1], op=ALU.mult)
            nc.vector.tensor_sub(out=gstats[:, 1:2], in0=gstats[:, 1:2], in1=tmp1[:])
            nc.scalar.activation(out=gstats[:, 1:2], in_=gstats[:, 1:2], func=FT.Sqrt,
                                 bias=eps_sb[0:16, :])
            nc.vector.reciprocal(out=gstats[:, 1:2], in_=gstats[:, 1:2])
            gstats_r = st.tile([16, 2], f32r, name=f"gstats_r_{i}", tag="gstats_r")
            nc.vector.tensor_copy(out=gstats_r[:], in_=gstats[:])
            # broadcast to channels: [128, 2] = (mean_c, rstd_c)
            ch_ps = ps.tile([128, 2], f32, name=f"ch_ps_{i}", tag="sc1")
            nc.tensor.matmul(out=ch_ps[:], lhsT=gselt_sb[:], rhs=gstats_r[:], start=True, stop=True)
            ab_i = st.tile([128, 2], f32, name=f"ab_{i}", tag="ab", bufs=2)
            nc.vector.tensor_tensor(out=ab_i[:, 0:1], in0=ch_ps[:, 1:2], in1=nw_sb[:, i:i + 1], op=ALU.mult)
            tmp2 = st.tile([128, 1], f32, name=f"tmp2_{i}", tag="tmp2")
            nc.vector.tensor_tensor(out=tmp2[:], in0=ch_ps[:, 0:1], in1=ab_i[:, 0:1], op=ALU.mult)
            nc.vector.tensor_sub(out=ab_i[:, 1:2], in0=nb_sb[:, i:i + 1], in1=tmp2[:])
            hp.__exit__(None, None, None)
            ab.append(ab_i)

        # preload the Exp activation table so the first attention exp
        # doesn't pay the table swap mid-stream
        exp_warm = st.tile([16, 1], f32, name="exp_warm", tag="expw")
        nc.scalar.activation(out=exp_warm[:], in_=eps_sb[0:16, :], func=FT.Exp)

        # apply affine -> xn (f16): c-tile 0 on DVE, c-tile 1 on ACT
        for i in range(2):
            for c2 in range(2):
                if i == 0:
                    nc.vector.tensor_scalar(
                        out=xn[:, i, c2 * 2048:(c2 + 1) * 2048],
                        in0=x_sb[:, i, c2 * 2048:(c2 + 1) * 2048],
                        scalar1=ab[i][:, 0:1], scalar2=ab[i][:, 1:2],
                        op0=ALU.mult, op1=ALU.add,
                    )
                else:
                    nc.scalar.activation(
                        out=xn[:, i, c2 * 2048:(c2 + 1) * 2048],
                        in_=x_sb[:, i, c2 * 2048:(c2 + 1) * 2048],
                        func=FT.Identity,
                        scale=ab[i][:, 0:1], bias=ab[i][:, 1:2],
                    )

        # ---- phase B: qkv projections ----
        # PE warm-up: ~5us of dummy matmuls so HAM reaches K=8/8 before the
        # dense qkv/attention stream (output is discarded).
        warm_ps = ps.tile([128, 512], f32, name="warm_ps", tag="sc0")
        for _ in range(48):
            nc.tensor.matmul(out=warm_ps[:, 0:128], lhsT=ones_sb[:], rhs=ones_sb[:],
                             start=True, stop=True)
        # q: [128, 2(pair), 1024]
        for p in range(2):
            q_ps = ps.tile([128, TS], f32, name=f"q_ps_{p}", tag=f"sc{p}")
            for nt in range(NT):
                for i in range(2):
                    nc.tensor.matmul(
                        out=q_ps[:, nt * 512:(nt + 1) * 512],
                        lhsT=qwt_sb[:, i, p * 128:(p + 1) * 128],
                        rhs=xn[:, i, nt * 512:(nt + 1) * 512],
                        start=(i == 0), stop=(i == 1),
                    )
            nc.vector.tensor_scalar_add(out=q_sb[:, p, :], in0=q_ps[:], scalar1=qb_sb[:, p:p + 1])
        # k and v^T production interleaved with attention consumption:
        # after chunk c4's k/v^T are emitted, attention js of chunk c4-1 for
        # pair 0 run, keeping ACT (exp) continuously busy from ~40us on.
        def k_thunks(c4):
            """Two emission units per pair: nt=0 matmuls, then nt=1 + bias copy."""
            units = []
            for p in range(2):
                cell = {}
                def mk_k0(p=p, cell=cell):
                    cell["t"] = ps.tile([128, 1024], f32, name=f"k_ps_{p}_{c4}", tag=f"sc{p}")
                    for i in range(2):
                        nc.tensor.matmul(
                            out=cell["t"][:, 0:512],
                            lhsT=kwt_sb[:, i, p * 128:(p + 1) * 128],
                            rhs=xn[:, i, c4 * 1024: c4 * 1024 + 512],
                            start=(i == 0), stop=(i == 1),
                        )
                def mk_k1(p=p, cell=cell):
                    for i in range(2):
                        nc.tensor.matmul(
                            out=cell["t"][:, 512:1024],
                            lhsT=kwt_sb[:, i, p * 128:(p + 1) * 128],
                            rhs=xn[:, i, c4 * 1024 + 512: c4 * 1024 + 1024],
                            start=(i == 0), stop=(i == 1),
                        )
                    nc.vector.tensor_scalar_add(
                        out=k_sb[:, p, c4 * 1024:(c4 + 1) * 1024], in0=cell["t"][:],
                        scalar1=kb_sb[:, p:p + 1],
                    )
                units += [mk_k0, mk_k1]
            return units

        def v_thunk(j):
            def mk_v(j=j):
                vt_ps = ps.tile([128, C], f32, name=f"vt_ps_{j}", tag=f"sc{j % 2}")
                for i in range(2):
                    nc.tensor.matmul(
                        out=vt_ps[:], lhsT=xn[:, i, j * 128:(j + 1) * 128],
                        rhs=vwt_sb[:, i, :], start=(i == 0), stop=(i == 1),
                    )
                nc.vector.tensor_copy(
                    out=vaug[:, j, :, 0:CH],
                    in_=vt_ps.rearrange("p (h c) -> p h c", h=HEADS),
                )
            return mk_v

        att = {}  # per-pair attention state: (avs, prev_w)
        att = {}  # per-pair attention state: (avs, prev_w)

        def emit_att(p, js, side=None):
            avs, prev_w = att[p]
            side = list(side or [])
            si = 0
            per_j = max(1, (len(side) + len(js) - 1) // len(js)) if side else 0
            for j in js:
                cur_w = [None, None]
                for hh in range(2):
                    h = 2 * p + hh
                    off = hh * CH
                    s_ps = ps.tile([128, TS], f32, name=f"s_ps_{h}_{j}", tag=f"sc{hh}")
                    for nt in range(NT):
                        nc.tensor.matmul(
                            out=s_ps[:, nt * 512:(nt + 1) * 512],
                            lhsT=k_sb[off:off + CH, p, j * 128:(j + 1) * 128],
                            rhs=q_sb[off:off + CH, p, nt * 512:(nt + 1) * 512],
                            start=True, stop=True,
                        )
                    w_t = wp.tile([128, TS], f16, name=f"w_{h}_{j}", tag="w")
                    cur_w[hh] = w_t
                    nc.scalar.activation(out=w_t[:], in_=s_ps[:], func=FT.Exp,
                                         scale=EXP_SCALE)
                    if prev_w[hh] is not None:
                        for nt in range(NT):
                            nc.tensor.matmul(
                                out=avs[hh][:, nt * 512:(nt + 1) * 512],
                                lhsT=vaug[:, j - 1, h, :],
                                rhs=prev_w[hh][:, nt * 512:(nt + 1) * 512],
                                start=(j - 1 == 0), stop=False,
                            )
                if side and si < len(side):
                    for t in side[si:si + per_j]:
                        t()
                    si += per_j
                elif 2 <= j < SJ - 1:
                    for f in range(3):
                        nc.tensor.matmul(
                            out=avs[0][:, (f % 2) * 512:(f % 2 + 1) * 512],
                            lhsT=zer_sb[:], rhs=xn[:, 0, 0:512],
                            start=False, stop=False, skip_group_check=True,
                        )
                prev_w = cur_w
            for t in side[si:]:
                t()
            att[p] = (avs, prev_w)

        def finish_pair(p):
            avs, prev_w = att[p]
            for hh in range(2):
                h = 2 * p + hh
                for nt in range(NT):
                    nc.tensor.matmul(
                        out=avs[hh][:, nt * 512:(nt + 1) * 512],
                        lhsT=vaug[:, SJ - 1, h, :],
                        rhs=prev_w[hh][:, nt * 512:(nt + 1) * 512],
                        start=False, stop=True,
                    )
            for hh in range(2):
                h = 2 * p + hh
                off = hh * CH
                av = avs[hh]
                # normalize: a = av[0:CH] / av[CH]
                d16 = rp.tile([1, TS], f16, name=f"d16_{h}", tag="d16")
                nc.vector.tensor_copy(out=d16[:], in_=av[CH:CH + 1, :])
                drep_ps = ps.tile([CH, TS], f32, name=f"drep_ps_{h}", tag=f"sc{hh}")
                for nt in range(NT):
                    nc.tensor.matmul(
                        out=drep_ps[:, nt * 512:(nt + 1) * 512],
                        lhsT=ones_sb[0:1, 0:CH],
                        rhs=d16[:, nt * 512:(nt + 1) * 512],
                        start=True, stop=True,
                    )
                rrep_sb = rp.tile([CH, TS], f32, name=f"rrep_sb_{h}", tag="rrep")
                nc.vector.reciprocal_approx_fast(out=rrep_sb[:], in_=drep_ps[:])
                nc.vector.tensor_tensor(
                    out=a_sb[off:off + CH, p, :], in0=av[0:CH, :], in1=rrep_sb[:],
                    op=ALU.mult,
                )

        att[0] = ([psa.tile([CH + 1, TS], f32, name=f"av_{hh}", tag=f"acc{hh}")
                   for hh in range(2)], [None, None])
        for t in k_thunks(0):
            t()
        v_thunk(0)()
        v_thunk(1)()
        for c4 in range(4):
            side = [v_thunk(j) for j in range(8 * c4 + 2, min(8 * c4 + 10, SJ))]
            if c4 < 3:
                side += k_thunks(c4 + 1)
            emit_att(0, range(8 * c4, 8 * c4 + 8), side=side)
        finish_pair(0)
        att[1] = ([psa.tile([CH + 1, TS], f32, name=f"av_{2 + hh}", tag=f"acc{hh}")
                   for hh in range(2)], [None, None])
        emit_att(1, range(SJ))
        finish_pair(1)

        # ---- phase D: projection + residual ----
        for m in range(2):
            h_ps = ps.tile([128, TS], f32, name=f"h_ps_{m}", tag=f"sc{m}")
            for nt in range(NT):
                for i in range(2):
                    nc.tensor.matmul(
                        out=h_ps[:, nt * 512:(nt + 1) * 512],
                        lhsT=pwt_sb[:, i, m * 128:(m + 1) * 128],
                        rhs=a_sb[:, i, nt * 512:(nt + 1) * 512],
                        start=(i == 0), stop=(i == 1),
                    )
            o_sb = wp.tile([128, TS], f32, name=f"o_sb_{m}", tag="w")
            nc.vector.scalar_tensor_tensor(
                out=o_sb[:], in0=h_ps[:], scalar=pb_sb[:, m:m + 1], in1=x_sb[:, m, 0:TS],
                op0=ALU.add, op1=ALU.add,
            )
            nc.sync.dma_start(out=out[m * 128:(m + 1) * 128, :], in_=o_sb[:])

    nc.compile()
    return nc


def _host_inputs(x, norm_w, norm_b, qkv_w, qkv_b, proj_w, proj_b):
    """Build the 8 per-core input maps (all float32 numpy)."""
    x = np.ascontiguousarray(np.asarray(x, dtype=np.float32)).reshape(B, C, T)
    norm_w = np.asarray(norm_w, dtype=np.float32)
    norm_b = np.asarray(norm_b, dtype=np.float32)
    qkv_w = np.asarray(qkv_w, dtype=np.float32)
    qkv_b = np.asarray(qkv_b, dtype=np.float32)
    proj_w = np.asarray(proj_w, dtype=np.float32)
    proj_b = np.asarray(proj_b, dtype=np.float32)

    # head-major row gathers of the qkv conv
    q_rows = np.concatenate([np.arange(192 * h, 192 * h + 64) for h in range(HEADS)])
    k_rows = q_rows + 64
    v_rows = q_rows + 128
    qwt = np.ascontiguousarray(qkv_w[q_rows].T.astype(np.float16))
    kwt = np.ascontiguousarray(qkv_w[k_rows].T.astype(np.float16))
    vwt = np.ascontiguousarray(qkv_w[v_rows].T.astype(np.float16))
    pwt = np.ascontiguousarray(proj_w.T.astype(np.float16))

    def as2(v):  # (256,) -> [128, 2] with column p = channels 128p..128p+128
        return np.ascontiguousarray(v.reshape(2, 128).T)

    qb2 = as2(qkv_b[q_rows])
    kb2 = as2(qkv_b[k_rows])
    # v bias folded into projection bias (a_norm lacks +vb; h += proj_w @ vb)
    vb_nat = qkv_b[v_rows]  # natural channel order == head-major for v
    pb2 = as2(proj_b + proj_w @ vb_nat)
    nw2 = as2(norm_w)
    nb2 = as2(norm_b)

    gsel = np.zeros((128, 16), np.float32)
    gsel[np.arange(128), np.arange(128) // 8] = 1.0
    gselt = np.ascontiguousarray(gsel.T)
    ones = np.ones((128, 128), np.float16)

    shared = dict(qwt=qwt, kwt=kwt, vwt=vwt, pwt=pwt, qb2=qb2, kb2=kb2,
                  pb2=pb2, nw2=nw2, nb2=nb2, gsel=gsel, gselt=gselt, ones=ones,
                  zer=np.zeros((128, 65), np.float16))
    in_maps = []
    for core in range(N_CORES):
        b, j = core // 4, core % 4
        xr = np.concatenate([x[b][:, j * TS:], x[b][:, :j * TS]], axis=1)
        in_maps.append({"x": np.ascontiguousarray(xr), **shared})
    return in_maps


def _run(in_maps, **kw):
    if "nc" not in _CACHE:
        _CACHE["nc"] = _build()
    return run_bass_kernel_spmd(_CACHE["nc"], in_maps, list(range(N_CORES)), **kw)


def kernel(x, norm_w, norm_b, qkv_w, qkv_b, proj_w, proj_b):
    in_maps = _host_inputs(x, norm_w, norm_b, qkv_w, qkv_b, proj_w, proj_b)
    res = _run(in_maps)
    out = np.empty((B, C, T), np.float32)
    for core in range(N_CORES):
        b, j = core // 4, core % 4
        out[b][:, j * TS:(j + 1) * TS] = res.results[core]["out"]
    return out.reshape(B, C, HH, WW)
